# revision 45
# baseline (speedup 1.0000x reference)
"""GCN forward (gather + segment-sum + matmul) on 8 TRN2 NeuronCores.

Algorithm (factorized GCN):
    out[i] = deg[i] * (sum_{j in N(i)} deg[j] * X[j]) @ W

Sharding: destination nodes are split across the 8 cores (12500 rows each);
the deg_src-prescaled fp16 feature table X' = deg[:,None]*X is replicated to
every core's HBM. Each core:
  - bin-packs its 12500 dests into 100 windows of <=128 so every
    (chunk,window) cell holds <=512 edges = exactly 4 gather tiles
    (~1.4% tile padding vs ~25% for contiguous windows),
  - gathers the fp16 rows of X' for its ~200K edges with gpsimd dma_gather
    on 4 SWDGE queues (the memory-bound part; int16 gather indices force a
    4-way chunking of the 100K-row table, so each core keeps 4 chunk-local
    edge streams, each cell's edges sorted by source for HBM locality),
  - builds one-hot matrices sel[e,d] = (dstrel[e] == d) in batches of 16
    tiles with a single broadcast-AP DVE is_equal,
  - segment-sums via TensorE: A_T[f,d] += G[e,f]^T @ sel[e,d], accumulating
    in PSUM over a window's edge tiles round-robined across the 4 chunk
    streams,
  - applies W with a second matmul and scales rows by deg_dest into a
    persistent SBUF output buffer, written back with one contiguous DMA,
  - the host inverse-permutes the rows (window packing) and concatenates.

The per-edge aggregation, both matmuls and the deg_dest scaling run on
device; the host computes indices/partitioning and stages dtype-converted,
deg_src-prescaled inputs.
"""
import os

import numpy as np

N = 100000
E = 1600000
F = 128
P = 128
NCORES = 8
NPC = N // NCORES          # 12500 destination rows per core
# 100 windows of <=128 destinations per core: two more than the minimum 98 so
# the per-(chunk,window) average load (~500) sits below the 4-tile cap (512)
# with slack for the bin-packing to hit it
NW = 100
NQ = 4                     # table chunks (int16 gather indices)
CHUNK = 25000              # rows per chunk
# tiles per gather call, staggered per queue so the 4 queues' gen/drain
# phases decorrelate (lockstep leaves the SDMA engines idle during the
# synchronized desc-gen+completion gaps). Per-call num_idxs is capped by the
# SWDGE descriptor-ring carveout (~96 descs/lane; 97 crashed the device),
# so 12 tiles (96/lane) is the largest safe call.
_gbt_env = os.environ.get("GCN_GB_TILES", "12")
GBT = [int(x) for x in (_gbt_env.split(",") * 4)[:4]] if "," in _gbt_env \
    else [int(_gbt_env)] * 4
GB_TILES = max(GBT)

_PROGRAM_CACHE: dict = {}


def _row_ids_from_pointers(row_pointers: np.ndarray) -> np.ndarray:
    """Replicates jnp.repeat(arange(N), diff(rp), total_repeat_length=E)."""
    rl = np.diff(row_pointers.astype(np.int64))
    starts = np.concatenate([np.zeros(1, np.int64), np.cumsum(rl)[:-1]])
    return np.searchsorted(starts, np.arange(E, dtype=np.int64), side="right") - 1


def _group_dests(cnt):
    """Bin-pack one core's destinations into NW windows of <=128 dests.

    cnt: [NPC, NQ] per-dest per-chunk edge counts. Windows 2..NW-1 are
    hard-capped at 4*P edges per chunk (exactly 4 gather tiles); windows
    0-1 absorb the overflow. Any grouping is correct (t_qw is computed
    from actual counts); the caps only minimize tile padding.
    Returns (grp [NPC], pos [NPC]).
    """
    CAP = 4 * P
    NREG = NW - 2
    loads = np.zeros((NREG, NQ), np.int64)
    sizes = np.zeros(NREG, np.int64)
    grp = np.full(NPC, -1, np.int64)
    order = np.argsort(-cnt.max(axis=1), kind="stable")
    overflow = []
    big = 1 << 40
    for d in order:
        v = cnt[d]
        cand = loads + v
        score = cand.max(axis=1)
        score[(cand > CAP).any(axis=1) | (sizes >= P)] = big
        g = int(np.argmin(score))
        if score[g] >= big:
            overflow.append(d)
            continue
        loads[g] = cand[g]
        sizes[g] += 1
        grp[d] = g + 2
    # overflow windows 0/1: size-capped only
    osz = [0, 0]
    for d in overflow:
        g = 0 if osz[0] <= osz[1] and osz[0] < P else 1
        if osz[g] >= P:  # both full: spill into least-loaded regular window
            g2 = int(np.argmin(np.where(sizes < P, loads.max(axis=1), big)))
            loads[g2] += cnt[d]
            sizes[g2] += 1
            grp[d] = g2 + 2
            continue
        osz[g] += 1
        grp[d] = g
    # positions within each window
    pos = np.zeros(NPC, np.int64)
    nxt = np.zeros(NW, np.int64)
    for d in range(NPC):
        g = grp[d]
        pos[d] = nxt[g]
        nxt[g] += 1
    return grp, pos


def _preprocess(X, weight, degrees, row_pointers, column_index):
    row_ids = _row_ids_from_pointers(row_pointers)          # [E] sorted, in [0,N)
    col = column_index.astype(np.int64)
    deg = np.ascontiguousarray(degrees.astype(np.float32))

    core = row_ids // NPC                                   # [E] in [0,8)
    local = row_ids - core * NPC
    q = col // CHUNK                                        # [E] in [0,4)
    src16_all = (col - q * CHUNK).astype(np.int16)

    # balanced dest->window grouping per core (kills tile padding)
    w_local = np.empty(E, np.int64)
    dstrel_all = np.empty(E, np.float32)
    dest_of = np.full((NCORES, NW * P), -1, np.int64)       # (c, w*128+p) -> node id
    for c in range(NCORES):
        m = core == c
        cnt = np.bincount(local[m] * NQ + q[m], minlength=NPC * NQ).reshape(NPC, NQ)
        grp, pos = _group_dests(cnt)
        w_local[m] = grp[local[m]]
        dstrel_all[m] = pos[local[m]]
        dest_of[c, grp * P + pos] = c * NPC + np.arange(NPC)

    key = ((core * NQ + q) * NW + w_local).astype(np.int64)  # (c, q, w)
    counts = np.bincount(key, minlength=NCORES * NQ * NW).reshape(NCORES, NQ, NW)
    t_qw = -(-counts.max(axis=0) // P)                       # [NQ, NW]
    # no chunk may have an empty stream (zero-size params break AP lowering);
    # a pad tile (src=0, dstrel=-1) contributes nothing
    for qq in range(NQ):
        if t_qw[qq].sum() == 0:
            t_qw[qq, 0] = 1
    lq = t_qw.sum(axis=1) * P                                # [NQ] stream lengths
    chunk_base = np.concatenate([np.zeros(1, np.int64), np.cumsum(lq)])
    ltot = int(chunk_base[-1])
    # offset of window w's padded segment within chunk q's stream
    offs_qw = np.cumsum(np.concatenate([np.zeros((NQ, 1), np.int64), t_qw[:, :-1]], axis=1) * P, axis=1) \
        if False else (np.cumsum(t_qw, axis=1) - t_qw) * P   # [NQ, NW] exclusive prefix

    # within each (core,chunk,window) cell, order edges by ascending source so
    # each SDMA engine's gather descriptors walk HBM mostly monotonically
    order = np.lexsort((src16_all, key))
    key_s = key[order]
    starts_flat = np.concatenate([np.zeros(1, np.int64), np.cumsum(counts.reshape(-1))])[:-1]
    rank_s = np.arange(E, dtype=np.int64) - starts_flat[key_s]
    q_s = (key_s // NW) % NQ
    w_s = key_s % NW
    core_s = key_s // (NQ * NW)
    pos_s = chunk_base[q_s] + offs_qw[q_s, w_s] + rank_s     # [E] position in core's array

    src_pad = np.zeros((NCORES, ltot), np.int16)
    dstrel_pad = np.full((NCORES, ltot), -1.0, np.float32)
    src_pad[core_s, pos_s] = src16_all[order]
    dstrel_pad[core_s, pos_s] = dstrel_all[order]

    # per-chunk device layouts
    idx_w, dst_t = [], []
    for qq in range(NQ):
        sl = slice(int(chunk_base[qq]), int(chunk_base[qq + 1]))
        s = src_pad[:, sl]                                   # [NC, LQ]
        # wrapped idx layout [128, LQ/16]: idx i at [i%16, i//16], replicated 8x
        iw = np.tile(s.reshape(NCORES, -1, 16).transpose(0, 2, 1), (1, 8, 1))
        idx_w.append(np.ascontiguousarray(iw))
        dst_t.append(np.ascontiguousarray(
            dstrel_pad[:, sl].reshape(NCORES, -1, P).transpose(0, 2, 1).astype(np.float16)))

    # per-core dest-degree table [P, NW] in (w, p) slot order
    degt = np.zeros((NCORES, P, NW), np.float32)
    for c in range(NCORES):
        ids = dest_of[c]                                     # [NW*P]
        dv = np.where(ids >= 0, deg[np.clip(ids, 0, N - 1)], 0.0)
        degt[c] = dv.reshape(NW, P).T

    # stage deg_src-prescaled features: the weighted segment-sum's per-edge
    # weights deg[col] fold into the gathered rows (host staging, like the
    # dtype conversion); the aggregation itself stays on device
    xt = np.ascontiguousarray((X.astype(np.float32) * deg[:, None]).astype(np.float16))
    w16 = np.ascontiguousarray(weight.astype(np.float16))
    t_key = tuple(tuple(int(x) for x in row) for row in t_qw)
    return xt, w16, idx_w, dst_t, degt, dest_of, t_key


SB_T = int(os.environ.get("GCN_SB_T", "16"))  # tiles per batched sel build


def _build_program(t_qw):
    import concourse.bacc as bacc
    import concourse.bass as bass
    import concourse.mybir as mybir
    import concourse.tile as tile

    lq = [sum(t_qw[q]) * P for q in range(NQ)]

    nc = bacc.Bacc("TRN2", target_bir_lowering=False, num_swdge_queues=4)
    xt_p = nc.declare_dram_parameter("xt", [N, F], mybir.dt.float16, isOutput=False)
    idx_ps = [nc.declare_dram_parameter(f"idx{q}", [P, lq[q] // 16], mybir.dt.int16, isOutput=False) for q in range(NQ)]
    dst_ps = [nc.declare_dram_parameter(f"dstrel{q}", [P, lq[q] // P], mybir.dt.float16, isOutput=False) for q in range(NQ)]
    degt_p = nc.declare_dram_parameter("degt", [P, NW], mybir.dt.float32, isOutput=False)
    w_p = nc.declare_dram_parameter("w16", [F, F], mybir.dt.float16, isOutput=False)
    # transposed output layout: row p holds window-major features so the
    # final DMA is one contiguous 49KB-per-partition write (host unshuffles)
    out_p = nc.declare_dram_parameter("out", [P, NW * F], mybir.dt.float32, isOutput=True)

    def bcast_mid(ap, t):
        # [128, t] AP -> [128, t, F] with stride-0 inner (value per (p, tile))
        return bass.AP(ap.tensor, ap.offset, [ap.ap[0], [ap.ap[1][0], t], [0, F]])

    # prep/trigger split measured 5x SLOWER (per-call trigger+sem overhead in
    # Tile mode swamps the gen/drain overlap it buys); keep the blocking form
    use_prep = os.environ.get("GCN_PREP", "0") == "1"
    dma_sems = [nc.alloc_semaphore(f"swdge_dma{q}") for q in range(NQ)] if use_prep else None
    with tile.TileContext(nc) as tc:
        with (
            tc.tile_pool(name="persist", bufs=1) as persist,
            tc.tile_pool(name="gblk", bufs=int(os.environ.get("GCN_GBUFS", "6"))) as gpool,
            tc.tile_pool(name="selp", bufs=int(os.environ.get("GCN_SBUFS", "2"))) as selpool,
            tc.tile_pool(name="atsb", bufs=2) as atpool,
            tc.tile_pool(name="outsb", bufs=2) as outpool,
            tc.tile_pool(name="psum1", bufs=2, space="PSUM") as psum1,
            tc.tile_pool(name="psum2", bufs=2, space="PSUM") as psum2,
        ):
            idx_sb, dst_sb = [], []
            for q in range(NQ):
                t1 = persist.tile([P, lq[q] // 16], mybir.dt.int16, tag=f"idx{q}", name=f"idx{q}")
                nc.sync.dma_start(t1[:], idx_ps[q][:])
                idx_sb.append(t1)
                t2 = persist.tile([P, lq[q] // P], mybir.dt.float16, tag=f"dst{q}", name=f"dst{q}")
                nc.sync.dma_start(t2[:], dst_ps[q][:])
                dst_sb.append(t2)
            degt_sb = persist.tile([P, NW], mybir.dt.float32)
            nc.sync.dma_start(degt_sb[:], degt_p[:])
            w_sb = persist.tile([F, F], mybir.dt.float16)
            nc.sync.dma_start(w_sb[:], w_p[:])
            c_i32 = persist.tile([P, P], mybir.dt.int32)
            nc.gpsimd.iota(c_i32[:], pattern=[[1, P]], base=0, channel_multiplier=0)
            c_f16 = persist.tile([P, P], mybir.dt.float16)
            nc.vector.tensor_copy(c_f16[:], c_i32[:])

            outbuf = persist.tile([P, NW * F], mybir.dt.float32, name="outbuf")

            pos = [0] * NQ
            calls_done = [0] * NQ
            gblk = [None] * NQ
            selblk = [None] * NQ
            # stagger queue phases: shorten only the FIRST call per queue
            # (12,9,6,3 tiles) so the queues' gen/drain cycles decorrelate
            # while steady-state calls stay at the 12-tile ring maximum
            ph_step = int(os.environ.get("GCN_PHASE", "3"))
            blk_start = [0] * NQ
            blk_size = [max(GBT[q] - ph_step * q, 1) for q in range(NQ)]
            flushed = 0
            for w in range(NW):
                ntiles_w = sum(t_qw[q][w] for q in range(NQ))
                if ntiles_w == 0:
                    nc.vector.memset(outbuf[:, w * F : (w + 1) * F], 0.0)
                    continue
                at_ps = psum1.tile([F, P], mybir.dt.float32, space="PSUM")
                k = 0
                # round-robin the window's tiles across the 4 chunk streams so
                # the queues' gather buffers free at an even pace
                rr = [q for t in range(max(t_qw[q][w] for q in range(NQ)))
                      for q in range(NQ) if t < t_qw[q][w]]
                if os.environ.get("GCN_RR", "1") != "1":
                    rr = [q for q in range(NQ) for _ in range(t_qw[q][w])]
                for q in rr:
                        if pos[q] == blk_start[q] + blk_size[q] or pos[q] == 0:
                            if pos[q] > 0:
                                blk_start[q] = pos[q]
                                blk_size[q] = GBT[q]
                            nt_call = min(blk_size[q], lq[q] // P - pos[q])
                            nidx = nt_call * P
                            gblk[q] = gpool.tile(
                                [P, GBT[q] * F], mybir.dt.float16,
                                tag=f"gblk{q}", name=f"gblk{q}",
                            )
                            gather_kw = dict(
                                out_ap=gblk[q][:, : nt_call * F].rearrange(
                                    "p (k f) -> p k f", f=F
                                ),
                                in_ap=xt_p[q * CHUNK : (q + 1) * CHUNK, :],
                                idxs_ap=idx_sb[q][:, pos[q] * P // 16 : (pos[q] * P + nidx) // 16],
                                num_idxs=nidx,
                                num_idxs_reg=nidx,
                                elem_size=F,
                                queue_num=q,
                                single_packet=(os.environ.get('GCN_SP','0')=='1'),
                            )
                            if use_prep:
                                # split prep/trigger: desc-gen overlaps the
                                # previous call's drain; consumers are gated
                                # by an explicit wait on the DMA sem fused
                                # into the block's first matmul
                                nc.gpsimd.dma_gather(
                                    prepare_only=True, sem=dma_sems[q], **gather_kw
                                )
                                nc.gpsimd.trigger_dma(count=None, queue_num=q)
                                calls_done[q] += 1
                            else:
                                nc.gpsimd.dma_gather(**gather_kw)
                        if pos[q] % SB_T == 0:
                            nt_s = min(SB_T, lq[q] // P - pos[q])
                            selblk[q] = selpool.tile(
                                [P, SB_T * F], mybir.dt.float16,
                                tag=f"sel{q}", name=f"sel{q}",
                            )
                            c_b = bass.AP(c_f16[:].tensor, c_f16[:].offset,
                                          [c_f16[:].ap[0], [0, nt_s], [1, F]])
                            nc.vector.tensor_tensor(
                                out=selblk[q][:, : nt_s * F].rearrange("p (t f) -> p t f", f=F),
                                in0=c_b,
                                in1=bcast_mid(dst_sb[q][:, pos[q] : pos[q] + nt_s], nt_s),
                                op=mybir.AluOpType.is_equal,
                            )
                        j = pos[q] - blk_start[q]
                        js = pos[q] % SB_T
                        if use_prep and j == 0:
                            # gate the block's first consumer on DMA landed
                            nc.tensor.wait_ge(dma_sems[q], 16 * calls_done[q])
                        nc.tensor.matmul(
                            out=at_ps[:],
                            lhsT=gblk[q][:, j * F : (j + 1) * F],
                            rhs=selblk[q][:, js * F : (js + 1) * F],
                            start=(k == 0),
                            stop=(k == ntiles_w - 1),
                        )
                        pos[q] += 1
                        k += 1
                at_sb = atpool.tile([F, P], mybir.dt.float16)
                nc.scalar.activation(at_sb[:], at_ps[:], mybir.ActivationFunctionType.Copy)
                o2_ps = psum2.tile([P, F], mybir.dt.float32, space="PSUM")
                nc.tensor.matmul(out=o2_ps[:], lhsT=at_sb[:], rhs=w_sb[:], start=True, stop=True)
                nc.scalar.activation(outbuf[:, w * F : (w + 1) * F], o2_ps[:],
                                     mybir.ActivationFunctionType.Copy,
                                     scale=degt_sb[:, w : w + 1])
                # flush finished quarters of the output buffer so the
                # writeback overlaps compute instead of trailing serially
                if (w + 1) % (NW // 4) == 0 and w + 1 < NW:
                    nc.sync.dma_start(out=out_p[:, flushed * F : (w + 1) * F],
                                      in_=outbuf[:, flushed * F : (w + 1) * F])
                    flushed = w + 1
            nc.sync.dma_start(out=out_p[:, flushed * F :], in_=outbuf[:, flushed * F :])
    nc.compile()
    return nc


def _get_program(t_key):
    key = (t_key, tuple(GBT), SB_T, os.environ.get("GCN_PHASE", "3"))
    if key not in _PROGRAM_CACHE:
        _PROGRAM_CACHE[key] = _build_program(t_key)
    return _PROGRAM_CACHE[key]


def _run(nc, in_maps, trace=False, **kw):
    from concourse.bass_utils import run_bass_kernel_spmd

    return run_bass_kernel_spmd(nc, in_maps, core_ids=list(range(NCORES)),
                                trace=trace, **kw)


def kernel(X, weight, degrees, row_pointers, column_index, _trace=False, _ret_raw=False):
    assert X.shape == (N, F) and column_index.shape == (E,)
    xt, w16, idx_w, dst_t, degt, dest_of, t_key = _preprocess(
        X, weight, degrees, row_pointers, column_index
    )
    nc = _get_program(t_key)
    in_maps = []
    for c in range(NCORES):
        m = {"xt": xt, "degt": degt[c], "w16": w16}
        for q in range(NQ):
            m[f"idx{q}"] = idx_w[q][c]
            m[f"dstrel{q}"] = dst_t[q][c]
        in_maps.append(m)
    res = _run(nc, in_maps, trace=_trace)
    out = np.empty((N, F), np.float32)
    for c in range(NCORES):
        r = res.results[c]["out"].reshape(P, NW, F).transpose(1, 0, 2).reshape(NW * P, F)
        ids = dest_of[c]
        valid = ids >= 0
        out[ids[valid]] = r[valid]
    if _ret_raw:
        return out, res
    return out



# revision 46
# speedup vs baseline: 1.1603x; 1.1603x over previous
"""GCN forward (gather + segment-sum + matmul) on 8 TRN2 NeuronCores.

Algorithm (factorized GCN):
    out[i] = deg[i] * (sum_{j in N(i)} deg[j] * X[j]) @ W

Sharding: destination nodes are split across the 8 cores (12500 rows each);
the deg_src-prescaled fp16 feature table X' = deg[:,None]*X is replicated to
every core's HBM. Each core:
  - bin-packs its 12500 dests into 100 windows of <=128 so every
    (chunk,window) cell holds <=512 edges = exactly 4 gather tiles
    (~1.4% tile padding vs ~25% for contiguous windows),
  - gathers the fp16 rows of X' for its ~200K edges with gpsimd dma_gather
    on 4 SWDGE queues (the memory-bound part; int16 gather indices force a
    4-way chunking of the 100K-row table, so each core keeps 4 chunk-local
    edge streams, each cell's edges sorted by source for HBM locality),
  - builds one-hot matrices sel[e,d] = (dstrel[e] == d) in batches of 16
    tiles with a single broadcast-AP DVE is_equal,
  - segment-sums via TensorE: A_T[f,d] += G[e,f]^T @ sel[e,d], accumulating
    in PSUM over a window's edge tiles round-robined across the 4 chunk
    streams,
  - applies W with a second matmul and scales rows by deg_dest into a
    persistent SBUF output buffer, written back with one contiguous DMA,
  - the host inverse-permutes the rows (window packing) and concatenates.

The per-edge aggregation, both matmuls and the deg_dest scaling run on
device; the host computes indices/partitioning and stages dtype-converted,
deg_src-prescaled inputs.
"""
import os

import numpy as np

N = 100000
E = 1600000
F = 128
P = 128
NCORES = 8
NPC = N // NCORES          # 12500 destination rows per core
# 100 windows of <=128 destinations per core: two more than the minimum 98 so
# the per-(chunk,window) average load (~500) sits below the 4-tile cap (512)
# with slack for the bin-packing to hit it
NW = 100
NQ = 4                     # table chunks (int16 gather indices)
CHUNK = 25000              # rows per chunk
# tiles per gather call, staggered per queue so the 4 queues' gen/drain
# phases decorrelate (lockstep leaves the SDMA engines idle during the
# synchronized desc-gen+completion gaps). Per-call num_idxs is capped by the
# SWDGE descriptor-ring carveout (~96 descs/lane; 97 crashed the device),
# so 12 tiles (96/lane) is the largest safe call.
_gbt_env = os.environ.get("GCN_GB_TILES", "12")
GBT = [int(x) for x in (_gbt_env.split(",") * 4)[:4]] if "," in _gbt_env \
    else [int(_gbt_env)] * 4
GB_TILES = max(GBT)

_PROGRAM_CACHE: dict = {}


def _row_ids_from_pointers(row_pointers: np.ndarray) -> np.ndarray:
    """Replicates jnp.repeat(arange(N), diff(rp), total_repeat_length=E)."""
    rl = np.diff(row_pointers.astype(np.int64))
    starts = np.concatenate([np.zeros(1, np.int64), np.cumsum(rl)[:-1]])
    return np.searchsorted(starts, np.arange(E, dtype=np.int64), side="right") - 1


def _group_dests(cnt):
    """Bin-pack one core's destinations into NW windows of <=128 dests.

    cnt: [NPC, NQ] per-dest per-chunk edge counts. Windows 2..NW-1 are
    hard-capped at 4*P edges per chunk (exactly 4 gather tiles); windows
    0-1 absorb the overflow. Any grouping is correct (t_qw is computed
    from actual counts); the caps only minimize tile padding.
    Returns (grp [NPC], pos [NPC]).
    """
    CAP = 4 * P
    NREG = NW - 2
    loads = np.zeros((NREG, NQ), np.int64)
    sizes = np.zeros(NREG, np.int64)
    grp = np.full(NPC, -1, np.int64)
    order = np.argsort(-cnt.max(axis=1), kind="stable")
    overflow = []
    big = 1 << 40
    for d in order:
        v = cnt[d]
        cand = loads + v
        score = cand.max(axis=1)
        score[(cand > CAP).any(axis=1) | (sizes >= P)] = big
        g = int(np.argmin(score))
        if score[g] >= big:
            overflow.append(d)
            continue
        loads[g] = cand[g]
        sizes[g] += 1
        grp[d] = g + 2
    # overflow windows 0/1: size-capped only
    osz = [0, 0]
    for d in overflow:
        g = 0 if osz[0] <= osz[1] and osz[0] < P else 1
        if osz[g] >= P:  # both full: spill into least-loaded regular window
            g2 = int(np.argmin(np.where(sizes < P, loads.max(axis=1), big)))
            loads[g2] += cnt[d]
            sizes[g2] += 1
            grp[d] = g2 + 2
            continue
        osz[g] += 1
        grp[d] = g
    # positions within each window
    pos = np.zeros(NPC, np.int64)
    nxt = np.zeros(NW, np.int64)
    for d in range(NPC):
        g = grp[d]
        pos[d] = nxt[g]
        nxt[g] += 1
    return grp, pos


def _preprocess(X, weight, degrees, row_pointers, column_index):
    row_ids = _row_ids_from_pointers(row_pointers)          # [E] sorted, in [0,N)
    col = column_index.astype(np.int64)
    deg = np.ascontiguousarray(degrees.astype(np.float32))

    core = row_ids // NPC                                   # [E] in [0,8)
    local = row_ids - core * NPC
    q = col // CHUNK                                        # [E] in [0,4)
    src16_all = (col - q * CHUNK).astype(np.int16)

    # balanced dest->window grouping per core (kills tile padding)
    w_local = np.empty(E, np.int64)
    dstrel_all = np.empty(E, np.float32)
    dest_of = np.full((NCORES, NW * P), -1, np.int64)       # (c, w*128+p) -> node id
    for c in range(NCORES):
        m = core == c
        cnt = np.bincount(local[m] * NQ + q[m], minlength=NPC * NQ).reshape(NPC, NQ)
        grp, pos = _group_dests(cnt)
        w_local[m] = grp[local[m]]
        dstrel_all[m] = pos[local[m]]
        dest_of[c, grp * P + pos] = c * NPC + np.arange(NPC)

    key = ((core * NQ + q) * NW + w_local).astype(np.int64)  # (c, q, w)
    counts = np.bincount(key, minlength=NCORES * NQ * NW).reshape(NCORES, NQ, NW)
    t_qw = -(-counts.max(axis=0) // P)                       # [NQ, NW]
    # no chunk may have an empty stream (zero-size params break AP lowering);
    # a pad tile (src=0, dstrel=-1) contributes nothing
    for qq in range(NQ):
        if t_qw[qq].sum() == 0:
            t_qw[qq, 0] = 1
    lq = t_qw.sum(axis=1) * P                                # [NQ] stream lengths
    chunk_base = np.concatenate([np.zeros(1, np.int64), np.cumsum(lq)])
    ltot = int(chunk_base[-1])
    # offset of window w's padded segment within chunk q's stream
    offs_qw = np.cumsum(np.concatenate([np.zeros((NQ, 1), np.int64), t_qw[:, :-1]], axis=1) * P, axis=1) \
        if False else (np.cumsum(t_qw, axis=1) - t_qw) * P   # [NQ, NW] exclusive prefix

    # within each (core,chunk,window) cell, order edges by ascending source so
    # each SDMA engine's gather descriptors walk HBM mostly monotonically
    order = np.lexsort((src16_all, key))
    key_s = key[order]
    starts_flat = np.concatenate([np.zeros(1, np.int64), np.cumsum(counts.reshape(-1))])[:-1]
    rank_s = np.arange(E, dtype=np.int64) - starts_flat[key_s]
    q_s = (key_s // NW) % NQ
    w_s = key_s % NW
    core_s = key_s // (NQ * NW)
    pos_s = chunk_base[q_s] + offs_qw[q_s, w_s] + rank_s     # [E] position in core's array

    src_pad = np.zeros((NCORES, ltot), np.int16)
    dstrel_pad = np.full((NCORES, ltot), -1.0, np.float32)
    src_pad[core_s, pos_s] = src16_all[order]
    dstrel_pad[core_s, pos_s] = dstrel_all[order]

    # per-chunk device layouts
    idx_w, dst_t = [], []
    for qq in range(NQ):
        sl = slice(int(chunk_base[qq]), int(chunk_base[qq + 1]))
        s = src_pad[:, sl]                                   # [NC, LQ]
        # wrapped idx layout [128, LQ/16]: idx i at [i%16, i//16], replicated 8x
        iw = np.tile(s.reshape(NCORES, -1, 16).transpose(0, 2, 1), (1, 8, 1))
        idx_w.append(np.ascontiguousarray(iw))
        dst_t.append(np.ascontiguousarray(
            dstrel_pad[:, sl].reshape(NCORES, -1, P).transpose(0, 2, 1).astype(np.float16)))

    # per-core dest-degree table [P, NW] in (w, p) slot order
    degt = np.zeros((NCORES, P, NW), np.float32)
    for c in range(NCORES):
        ids = dest_of[c]                                     # [NW*P]
        dv = np.where(ids >= 0, deg[np.clip(ids, 0, N - 1)], 0.0)
        degt[c] = dv.reshape(NW, P).T

    # stage deg_src-prescaled features: the weighted segment-sum's per-edge
    # weights deg[col] fold into the gathered rows (host staging, like the
    # dtype conversion); the aggregation itself stays on device
    xt = np.ascontiguousarray((X.astype(np.float32) * deg[:, None]).astype(np.float16))
    w16 = np.ascontiguousarray(weight.astype(np.float16))
    t_key = tuple(tuple(int(x) for x in row) for row in t_qw)
    return xt, w16, idx_w, dst_t, degt, dest_of, t_key


SB_T = int(os.environ.get("GCN_SB_T", "16"))  # tiles per batched sel build


def _build_program(t_qw):
    import concourse.bacc as bacc
    import concourse.bass as bass
    import concourse.mybir as mybir
    import concourse.tile as tile

    lq = [sum(t_qw[q]) * P for q in range(NQ)]

    nc = bacc.Bacc("TRN2", target_bir_lowering=False, num_swdge_queues=4)
    xt_p = nc.declare_dram_parameter("xt", [N, F], mybir.dt.float16, isOutput=False)
    idx_ps = [nc.declare_dram_parameter(f"idx{q}", [P, lq[q] // 16], mybir.dt.int16, isOutput=False) for q in range(NQ)]
    dst_ps = [nc.declare_dram_parameter(f"dstrel{q}", [P, lq[q] // P], mybir.dt.float16, isOutput=False) for q in range(NQ)]
    degt_p = nc.declare_dram_parameter("degt", [P, NW], mybir.dt.float32, isOutput=False)
    w_p = nc.declare_dram_parameter("w16", [F, F], mybir.dt.float16, isOutput=False)
    # transposed output layout: row p holds window-major features so the
    # final DMA is one contiguous 49KB-per-partition write (host unshuffles)
    out_p = nc.declare_dram_parameter("out", [P, NW * F], mybir.dt.float32, isOutput=True)

    def bcast_mid(ap, t):
        # [128, t] AP -> [128, t, F] with stride-0 inner (value per (p, tile))
        return bass.AP(ap.tensor, ap.offset, [ap.ap[0], [ap.ap[1][0], t], [0, F]])

    # prep/trigger split measured 5x SLOWER (per-call trigger+sem overhead in
    # Tile mode swamps the gen/drain overlap it buys); keep the blocking form
    use_prep = os.environ.get("GCN_PREP", "0") == "1"
    dma_sems = [nc.alloc_semaphore(f"swdge_dma{q}") for q in range(NQ)] if use_prep else None
    with tile.TileContext(nc) as tc:
        with (
            tc.tile_pool(name="persist", bufs=1) as persist,
            tc.tile_pool(name="gblk", bufs=int(os.environ.get("GCN_GBUFS", "4"))) as gpool,
            tc.tile_pool(name="selp", bufs=int(os.environ.get("GCN_SBUFS", "2"))) as selpool,
            tc.tile_pool(name="atsb", bufs=2) as atpool,
            tc.tile_pool(name="outsb", bufs=2) as outpool,
            tc.tile_pool(name="psum1", bufs=2, space="PSUM") as psum1,
            tc.tile_pool(name="psum2", bufs=2, space="PSUM") as psum2,
        ):
            idx_sb, dst_sb = [], []
            for q in range(NQ):
                t1 = persist.tile([P, lq[q] // 16], mybir.dt.int16, tag=f"idx{q}", name=f"idx{q}")
                nc.sync.dma_start(t1[:], idx_ps[q][:])
                idx_sb.append(t1)
                t2 = persist.tile([P, lq[q] // P], mybir.dt.float16, tag=f"dst{q}", name=f"dst{q}")
                nc.sync.dma_start(t2[:], dst_ps[q][:])
                dst_sb.append(t2)
            degt_sb = persist.tile([P, NW], mybir.dt.float32)
            nc.sync.dma_start(degt_sb[:], degt_p[:])
            w_sb = persist.tile([F, F], mybir.dt.float16)
            nc.sync.dma_start(w_sb[:], w_p[:])
            c_i32 = persist.tile([P, P], mybir.dt.int32)
            nc.gpsimd.iota(c_i32[:], pattern=[[1, P]], base=0, channel_multiplier=0)
            c_f16 = persist.tile([P, P], mybir.dt.float16)
            nc.vector.tensor_copy(c_f16[:], c_i32[:])

            outbuf = persist.tile([P, NW * F], mybir.dt.float32, name="outbuf")

            pos = [0] * NQ
            calls_done = [0] * NQ
            gblk = [None] * NQ
            selblk = [None] * NQ
            # stagger queue phases: shorten only the FIRST call per queue
            # (12,9,6,3 tiles) so the queues' gen/drain cycles decorrelate
            # while steady-state calls stay at the 12-tile ring maximum
            ph_step = int(os.environ.get("GCN_PHASE", "3"))
            blk_start = [0] * NQ
            blk_size = [max(GBT[q] - ph_step * q, 1) for q in range(NQ)]
            for w in range(NW):
                ntiles_w = sum(t_qw[q][w] for q in range(NQ))
                if ntiles_w == 0:
                    nc.vector.memset(outbuf[:, w * F : (w + 1) * F], 0.0)
                    continue
                at_ps = psum1.tile([F, P], mybir.dt.float32, space="PSUM")
                k = 0
                # round-robin the window's tiles across the 4 chunk streams so
                # the queues' gather buffers free at an even pace
                rr = [q for t in range(max(t_qw[q][w] for q in range(NQ)))
                      for q in range(NQ) if t < t_qw[q][w]]
                if os.environ.get("GCN_RR", "1") != "1":
                    rr = [q for q in range(NQ) for _ in range(t_qw[q][w])]
                for q in rr:
                        if pos[q] == blk_start[q] + blk_size[q] or pos[q] == 0:
                            if pos[q] > 0:
                                blk_start[q] = pos[q]
                                blk_size[q] = GBT[q]
                            nt_call = min(blk_size[q], lq[q] // P - pos[q])
                            nidx = nt_call * P
                            gblk[q] = gpool.tile(
                                [P, GBT[q] * F], mybir.dt.float16,
                                tag=f"gblk{q}", name=f"gblk{q}",
                            )
                            gather_kw = dict(
                                out_ap=gblk[q][:, : nt_call * F].rearrange(
                                    "p (k f) -> p k f", f=F
                                ),
                                in_ap=xt_p[q * CHUNK : (q + 1) * CHUNK, :],
                                idxs_ap=idx_sb[q][:, pos[q] * P // 16 : (pos[q] * P + nidx) // 16],
                                num_idxs=nidx,
                                num_idxs_reg=nidx,
                                elem_size=F,
                                queue_num=q,
                                single_packet=(os.environ.get('GCN_SP','0')=='1'),
                            )
                            if use_prep:
                                # split prep/trigger: desc-gen overlaps the
                                # previous call's drain; consumers are gated
                                # by an explicit wait on the DMA sem fused
                                # into the block's first matmul
                                nc.gpsimd.dma_gather(
                                    prepare_only=True, sem=dma_sems[q], **gather_kw
                                )
                                nc.gpsimd.trigger_dma(count=None, queue_num=q)
                                calls_done[q] += 1
                            else:
                                nc.gpsimd.dma_gather(**gather_kw)
                        if pos[q] % SB_T == 0:
                            nt_s = min(SB_T, lq[q] // P - pos[q])
                            selblk[q] = selpool.tile(
                                [P, SB_T * F], mybir.dt.float16,
                                tag=f"sel{q}", name=f"sel{q}",
                            )
                            c_b = bass.AP(c_f16[:].tensor, c_f16[:].offset,
                                          [c_f16[:].ap[0], [0, nt_s], [1, F]])
                            nc.vector.tensor_tensor(
                                out=selblk[q][:, : nt_s * F].rearrange("p (t f) -> p t f", f=F),
                                in0=c_b,
                                in1=bcast_mid(dst_sb[q][:, pos[q] : pos[q] + nt_s], nt_s),
                                op=mybir.AluOpType.is_equal,
                            )
                        j = pos[q] - blk_start[q]
                        js = pos[q] % SB_T
                        if use_prep and j == 0:
                            # gate the block's first consumer on DMA landed
                            nc.tensor.wait_ge(dma_sems[q], 16 * calls_done[q])
                        nc.tensor.matmul(
                            out=at_ps[:],
                            lhsT=gblk[q][:, j * F : (j + 1) * F],
                            rhs=selblk[q][:, js * F : (js + 1) * F],
                            start=(k == 0),
                            stop=(k == ntiles_w - 1),
                        )
                        pos[q] += 1
                        k += 1
                at_sb = atpool.tile([F, P], mybir.dt.float16)
                nc.scalar.activation(at_sb[:], at_ps[:], mybir.ActivationFunctionType.Copy)
                o2_ps = psum2.tile([P, F], mybir.dt.float32, space="PSUM")
                nc.tensor.matmul(out=o2_ps[:], lhsT=at_sb[:], rhs=w_sb[:], start=True, stop=True)
                nc.scalar.activation(outbuf[:, w * F : (w + 1) * F], o2_ps[:],
                                     mybir.ActivationFunctionType.Copy,
                                     scale=degt_sb[:, w : w + 1])
            nc.sync.dma_start(out=out_p[:], in_=outbuf[:])
    nc.compile()
    return nc


def _get_program(t_key):
    key = (t_key, tuple(GBT), SB_T, os.environ.get("GCN_PHASE", "3"))
    if key not in _PROGRAM_CACHE:
        _PROGRAM_CACHE[key] = _build_program(t_key)
    return _PROGRAM_CACHE[key]


def _run(nc, in_maps, trace=False, **kw):
    from concourse.bass_utils import run_bass_kernel_spmd

    return run_bass_kernel_spmd(nc, in_maps, core_ids=list(range(NCORES)),
                                trace=trace, **kw)


def kernel(X, weight, degrees, row_pointers, column_index, _trace=False, _ret_raw=False):
    assert X.shape == (N, F) and column_index.shape == (E,)
    xt, w16, idx_w, dst_t, degt, dest_of, t_key = _preprocess(
        X, weight, degrees, row_pointers, column_index
    )
    nc = _get_program(t_key)
    in_maps = []
    for c in range(NCORES):
        m = {"xt": xt, "degt": degt[c], "w16": w16}
        for q in range(NQ):
            m[f"idx{q}"] = idx_w[q][c]
            m[f"dstrel{q}"] = dst_t[q][c]
        in_maps.append(m)
    res = _run(nc, in_maps, trace=_trace)
    out = np.empty((N, F), np.float32)
    for c in range(NCORES):
        r = res.results[c]["out"].reshape(P, NW, F).transpose(1, 0, 2).reshape(NW * P, F)
        ids = dest_of[c]
        valid = ids >= 0
        out[ids[valid]] = r[valid]
    if _ret_raw:
        return out, res
    return out



# revision 48
# speedup vs baseline: 1.2055x; 1.0390x over previous
"""GCN forward (gather + segment-sum + matmul) on 8 TRN2 NeuronCores.

Algorithm (factorized GCN):
    out[i] = deg[i] * (sum_{j in N(i)} deg[j] * X[j]) @ W

Sharding: destination nodes are split across the 8 cores (12500 rows each);
the deg_src-prescaled fp16 feature table X' = deg[:,None]*X is replicated to
every core's HBM. Each core:
  - bin-packs its 12500 dests into 100 windows of <=128 so every
    (chunk,window) cell holds <=512 edges = exactly 4 gather tiles
    (~1.4% tile padding vs ~25% for contiguous windows),
  - gathers the fp16 rows of X' for its ~200K edges with gpsimd dma_gather
    on 4 SWDGE queues (the memory-bound part; int16 gather indices force a
    4-way chunking of the 100K-row table, so each core keeps 4 chunk-local
    edge streams, each cell's edges sorted by source for HBM locality),
  - builds one-hot matrices sel[e,d] = (dstrel[e] == d) in batches of 16
    tiles with a single broadcast-AP DVE is_equal,
  - segment-sums via TensorE: A_T[f,d] += G[e,f]^T @ sel[e,d], accumulating
    in PSUM over a window's edge tiles round-robined across the 4 chunk
    streams,
  - applies W with a second matmul and scales rows by deg_dest into a
    persistent SBUF output buffer, written back with one contiguous DMA,
  - the host inverse-permutes the rows (window packing) and concatenates.

The per-edge aggregation, both matmuls and the deg_dest scaling run on
device; the host computes indices/partitioning and stages dtype-converted,
deg_src-prescaled inputs.
"""
import os

import numpy as np

N = 100000
E = 1600000
F = 128
P = 128
NCORES = 8
NPC = N // NCORES          # 12500 destination rows per core
# 100 windows of <=128 destinations per core: two more than the minimum 98 so
# the per-(chunk,window) average load (~500) sits below the 4-tile cap (512)
# with slack for the bin-packing to hit it
NW = 100
NQ = 4                     # table chunks (int16 gather indices)
CHUNK = 25000              # rows per chunk
# tiles per gather call, staggered per queue so the 4 queues' gen/drain
# phases decorrelate (lockstep leaves the SDMA engines idle during the
# synchronized desc-gen+completion gaps). Per-call num_idxs is capped by the
# SWDGE descriptor-ring carveout (~96 descs/lane; 97 crashed the device),
# so 12 tiles (96/lane) is the largest safe call.
_gbt_env = os.environ.get("GCN_GB_TILES", "12")
GBT = [int(x) for x in (_gbt_env.split(",") * 4)[:4]] if "," in _gbt_env \
    else [int(_gbt_env)] * 4
GB_TILES = max(GBT)

_PROGRAM_CACHE: dict = {}


def _row_ids_from_pointers(row_pointers: np.ndarray) -> np.ndarray:
    """Replicates jnp.repeat(arange(N), diff(rp), total_repeat_length=E)."""
    rl = np.diff(row_pointers.astype(np.int64))
    starts = np.concatenate([np.zeros(1, np.int64), np.cumsum(rl)[:-1]])
    return np.searchsorted(starts, np.arange(E, dtype=np.int64), side="right") - 1


def _group_dests(cnt):
    """Bin-pack one core's destinations into NW windows of <=128 dests.

    cnt: [NPC, NQ] per-dest per-chunk edge counts. Windows 2..NW-1 are
    hard-capped at 4*P edges per chunk (exactly 4 gather tiles); windows
    0-1 absorb the overflow. Any grouping is correct (t_qw is computed
    from actual counts); the caps only minimize tile padding.
    Returns (grp [NPC], pos [NPC]).
    """
    CAP = 4 * P
    NREG = NW - 2
    loads = np.zeros((NREG, NQ), np.int64)
    sizes = np.zeros(NREG, np.int64)
    grp = np.full(NPC, -1, np.int64)
    order = np.argsort(-cnt.max(axis=1), kind="stable")
    overflow = []
    big = 1 << 40
    for d in order:
        v = cnt[d]
        cand = loads + v
        score = cand.max(axis=1)
        score[(cand > CAP).any(axis=1) | (sizes >= P)] = big
        g = int(np.argmin(score))
        if score[g] >= big:
            overflow.append(d)
            continue
        loads[g] = cand[g]
        sizes[g] += 1
        grp[d] = g + 2
    # overflow windows 0/1: size-capped only
    osz = [0, 0]
    for d in overflow:
        g = 0 if osz[0] <= osz[1] and osz[0] < P else 1
        if osz[g] >= P:  # both full: spill into least-loaded regular window
            g2 = int(np.argmin(np.where(sizes < P, loads.max(axis=1), big)))
            loads[g2] += cnt[d]
            sizes[g2] += 1
            grp[d] = g2 + 2
            continue
        osz[g] += 1
        grp[d] = g
    # positions within each window
    pos = np.zeros(NPC, np.int64)
    nxt = np.zeros(NW, np.int64)
    for d in range(NPC):
        g = grp[d]
        pos[d] = nxt[g]
        nxt[g] += 1
    return grp, pos


def _preprocess(X, weight, degrees, row_pointers, column_index):
    row_ids = _row_ids_from_pointers(row_pointers)          # [E] sorted, in [0,N)
    col = column_index.astype(np.int64)
    deg = np.ascontiguousarray(degrees.astype(np.float32))

    core = row_ids // NPC                                   # [E] in [0,8)
    local = row_ids - core * NPC
    q = col // CHUNK                                        # [E] in [0,4)
    src16_all = (col - q * CHUNK).astype(np.int16)

    # balanced dest->window grouping per core (kills tile padding)
    w_local = np.empty(E, np.int64)
    dstrel_all = np.empty(E, np.float32)
    dest_of = np.full((NCORES, NW * P), -1, np.int64)       # (c, w*128+p) -> node id
    for c in range(NCORES):
        m = core == c
        cnt = np.bincount(local[m] * NQ + q[m], minlength=NPC * NQ).reshape(NPC, NQ)
        grp, pos = _group_dests(cnt)
        w_local[m] = grp[local[m]]
        dstrel_all[m] = pos[local[m]]
        dest_of[c, grp * P + pos] = c * NPC + np.arange(NPC)

    key = ((core * NQ + q) * NW + w_local).astype(np.int64)  # (c, q, w)
    counts = np.bincount(key, minlength=NCORES * NQ * NW).reshape(NCORES, NQ, NW)
    t_qw = -(-counts.max(axis=0) // P)                       # [NQ, NW]
    # no chunk may have an empty stream (zero-size params break AP lowering);
    # a pad tile (src=0, dstrel=-1) contributes nothing
    for qq in range(NQ):
        if t_qw[qq].sum() == 0:
            t_qw[qq, 0] = 1
    lq = t_qw.sum(axis=1) * P                                # [NQ] stream lengths
    chunk_base = np.concatenate([np.zeros(1, np.int64), np.cumsum(lq)])
    ltot = int(chunk_base[-1])
    # offset of window w's padded segment within chunk q's stream
    offs_qw = np.cumsum(np.concatenate([np.zeros((NQ, 1), np.int64), t_qw[:, :-1]], axis=1) * P, axis=1) \
        if False else (np.cumsum(t_qw, axis=1) - t_qw) * P   # [NQ, NW] exclusive prefix

    # within each (core,chunk,window) cell, order edges by ascending source so
    # each SDMA engine's gather descriptors walk HBM mostly monotonically
    order = np.lexsort((src16_all, key))
    key_s = key[order]
    starts_flat = np.concatenate([np.zeros(1, np.int64), np.cumsum(counts.reshape(-1))])[:-1]
    rank_s = np.arange(E, dtype=np.int64) - starts_flat[key_s]
    q_s = (key_s // NW) % NQ
    w_s = key_s % NW
    core_s = key_s // (NQ * NW)
    pos_s = chunk_base[q_s] + offs_qw[q_s, w_s] + rank_s     # [E] position in core's array

    src_pad = np.zeros((NCORES, ltot), np.int16)
    dstrel_pad = np.full((NCORES, ltot), -1.0, np.float32)
    src_pad[core_s, pos_s] = src16_all[order]
    dstrel_pad[core_s, pos_s] = dstrel_all[order]

    # per-chunk device layouts
    idx_w, dst_t = [], []
    for qq in range(NQ):
        sl = slice(int(chunk_base[qq]), int(chunk_base[qq + 1]))
        s = src_pad[:, sl]                                   # [NC, LQ]
        # wrapped idx layout [128, LQ/16]: idx i at [i%16, i//16], replicated 8x
        iw = np.tile(s.reshape(NCORES, -1, 16).transpose(0, 2, 1), (1, 8, 1))
        idx_w.append(np.ascontiguousarray(iw))
        dst_t.append(np.ascontiguousarray(
            dstrel_pad[:, sl].reshape(NCORES, -1, P).transpose(0, 2, 1).astype(np.float16)))

    # per-core dest-degree table [P, NW] in (w, p) slot order
    degt = np.zeros((NCORES, P, NW), np.float32)
    for c in range(NCORES):
        ids = dest_of[c]                                     # [NW*P]
        dv = np.where(ids >= 0, deg[np.clip(ids, 0, N - 1)], 0.0)
        degt[c] = dv.reshape(NW, P).T

    # stage deg_src-prescaled features: the weighted segment-sum's per-edge
    # weights deg[col] fold into the gathered rows (host staging, like the
    # dtype conversion); the aggregation itself stays on device
    xt = np.ascontiguousarray((X.astype(np.float32) * deg[:, None]).astype(np.float16))
    w16 = np.ascontiguousarray(weight.astype(np.float16))
    t_key = tuple(tuple(int(x) for x in row) for row in t_qw)
    return xt, w16, idx_w, dst_t, degt, dest_of, t_key


SB_T = int(os.environ.get("GCN_SB_T", "16"))  # tiles per batched sel build


def _build_program(t_qw):
    import concourse.bacc as bacc
    import concourse.bass as bass
    import concourse.mybir as mybir
    import concourse.tile as tile

    lq = [sum(t_qw[q]) * P for q in range(NQ)]

    nc = bacc.Bacc("TRN2", target_bir_lowering=False, num_swdge_queues=4)
    xt_p = nc.declare_dram_parameter("xt", [N, F], mybir.dt.float16, isOutput=False)
    idx_ps = [nc.declare_dram_parameter(f"idx{q}", [P, lq[q] // 16], mybir.dt.int16, isOutput=False) for q in range(NQ)]
    dst_ps = [nc.declare_dram_parameter(f"dstrel{q}", [P, lq[q] // P], mybir.dt.float16, isOutput=False) for q in range(NQ)]
    degt_p = nc.declare_dram_parameter("degt", [P, NW], mybir.dt.float32, isOutput=False)
    w_p = nc.declare_dram_parameter("w16", [F, F], mybir.dt.float16, isOutput=False)
    # transposed output layout: row p holds window-major features so the
    # final DMA is one contiguous 49KB-per-partition write (host unshuffles)
    out_p = nc.declare_dram_parameter("out", [P, NW * F], mybir.dt.float32, isOutput=True)

    def bcast_mid(ap, t):
        # [128, t] AP -> [128, t, F] with stride-0 inner (value per (p, tile))
        return bass.AP(ap.tensor, ap.offset, [ap.ap[0], [ap.ap[1][0], t], [0, F]])

    # prep/trigger split measured 5x SLOWER (per-call trigger+sem overhead in
    # Tile mode swamps the gen/drain overlap it buys); keep the blocking form
    use_prep = os.environ.get("GCN_PREP", "0") == "1"
    dma_sems = [nc.alloc_semaphore(f"swdge_dma{q}") for q in range(NQ)] if use_prep else None
    with tile.TileContext(nc) as tc:
        with (
            tc.tile_pool(name="persist", bufs=1) as persist,
            tc.tile_pool(name="gblk", bufs=int(os.environ.get("GCN_GBUFS", "4"))) as gpool,
            tc.tile_pool(name="selp", bufs=int(os.environ.get("GCN_SBUFS", "2"))) as selpool,
            tc.tile_pool(name="atsb", bufs=2) as atpool,
            tc.tile_pool(name="outsb", bufs=2) as outpool,
            tc.tile_pool(name="psum1", bufs=2, space="PSUM") as psum1,
            tc.tile_pool(name="psum2", bufs=2, space="PSUM") as psum2,
        ):
            idx_sb, dst_sb = [], []
            for q in range(NQ):
                t1 = persist.tile([P, lq[q] // 16], mybir.dt.int16, tag=f"idx{q}", name=f"idx{q}")
                nc.sync.dma_start(t1[:], idx_ps[q][:])
                idx_sb.append(t1)
                t2 = persist.tile([P, lq[q] // P], mybir.dt.float16, tag=f"dst{q}", name=f"dst{q}")
                nc.sync.dma_start(t2[:], dst_ps[q][:])
                dst_sb.append(t2)
            degt_sb = persist.tile([P, NW], mybir.dt.float32)
            nc.sync.dma_start(degt_sb[:], degt_p[:])
            w_sb = persist.tile([F, F], mybir.dt.float16)
            nc.sync.dma_start(w_sb[:], w_p[:])
            c_i32 = persist.tile([P, P], mybir.dt.int32)
            nc.gpsimd.iota(c_i32[:], pattern=[[1, P]], base=0, channel_multiplier=0)
            c_f16 = persist.tile([P, P], mybir.dt.float16)
            nc.vector.tensor_copy(c_f16[:], c_i32[:])

            outbuf = persist.tile([P, NW * F], mybir.dt.float32, name="outbuf")

            pos = [0] * NQ
            calls_done = [0] * NQ
            gblk = [None] * NQ
            selblk = [None] * NQ
            # stagger queue phases: shorten only the FIRST call per queue
            # (12,9,6,3 tiles) so the queues' gen/drain cycles decorrelate
            # while steady-state calls stay at the 12-tile ring maximum
            ph_step = int(os.environ.get("GCN_PHASE", "3"))
            blk_start = [0] * NQ
            blk_size = [max(GBT[q] - ph_step * q, 1) for q in range(NQ)]
            flushed = 0
            for w in range(NW):
                ntiles_w = sum(t_qw[q][w] for q in range(NQ))
                if ntiles_w == 0:
                    nc.vector.memset(outbuf[:, w * F : (w + 1) * F], 0.0)
                    continue
                at_ps = psum1.tile([F, P], mybir.dt.float32, space="PSUM")
                k = 0
                # round-robin the window's tiles across the 4 chunk streams so
                # the queues' gather buffers free at an even pace
                rr = [q for t in range(max(t_qw[q][w] for q in range(NQ)))
                      for q in range(NQ) if t < t_qw[q][w]]
                if os.environ.get("GCN_RR", "1") != "1":
                    rr = [q for q in range(NQ) for _ in range(t_qw[q][w])]
                for q in rr:
                        if pos[q] == blk_start[q] + blk_size[q] or pos[q] == 0:
                            if pos[q] > 0:
                                blk_start[q] = pos[q]
                                blk_size[q] = GBT[q]
                            nt_call = min(blk_size[q], lq[q] // P - pos[q])
                            nidx = nt_call * P
                            gblk[q] = gpool.tile(
                                [P, GBT[q] * F], mybir.dt.float16,
                                tag=f"gblk{q}", name=f"gblk{q}",
                            )
                            gather_kw = dict(
                                out_ap=gblk[q][:, : nt_call * F].rearrange(
                                    "p (k f) -> p k f", f=F
                                ),
                                in_ap=xt_p[q * CHUNK : (q + 1) * CHUNK, :],
                                idxs_ap=idx_sb[q][:, pos[q] * P // 16 : (pos[q] * P + nidx) // 16],
                                num_idxs=nidx,
                                num_idxs_reg=nidx,
                                elem_size=F,
                                queue_num=q,
                                single_packet=(os.environ.get('GCN_SP','0')=='1'),
                            )
                            if use_prep:
                                # split prep/trigger: desc-gen overlaps the
                                # previous call's drain; consumers are gated
                                # by an explicit wait on the DMA sem fused
                                # into the block's first matmul
                                nc.gpsimd.dma_gather(
                                    prepare_only=True, sem=dma_sems[q], **gather_kw
                                )
                                nc.gpsimd.trigger_dma(count=None, queue_num=q)
                                calls_done[q] += 1
                            else:
                                nc.gpsimd.dma_gather(**gather_kw)
                        if pos[q] % SB_T == 0:
                            nt_s = min(SB_T, lq[q] // P - pos[q])
                            selblk[q] = selpool.tile(
                                [P, SB_T * F], mybir.dt.float16,
                                tag=f"sel{q}", name=f"sel{q}",
                            )
                            c_b = bass.AP(c_f16[:].tensor, c_f16[:].offset,
                                          [c_f16[:].ap[0], [0, nt_s], [1, F]])
                            nc.vector.tensor_tensor(
                                out=selblk[q][:, : nt_s * F].rearrange("p (t f) -> p t f", f=F),
                                in0=c_b,
                                in1=bcast_mid(dst_sb[q][:, pos[q] : pos[q] + nt_s], nt_s),
                                op=mybir.AluOpType.is_equal,
                            )
                        j = pos[q] - blk_start[q]
                        js = pos[q] % SB_T
                        if use_prep and j == 0:
                            # gate the block's first consumer on DMA landed
                            nc.tensor.wait_ge(dma_sems[q], 16 * calls_done[q])
                        nc.tensor.matmul(
                            out=at_ps[:],
                            lhsT=gblk[q][:, j * F : (j + 1) * F],
                            rhs=selblk[q][:, js * F : (js + 1) * F],
                            start=(k == 0),
                            stop=(k == ntiles_w - 1),
                        )
                        pos[q] += 1
                        k += 1
                at_sb = atpool.tile([F, P], mybir.dt.float16)
                nc.scalar.activation(at_sb[:], at_ps[:], mybir.ActivationFunctionType.Copy)
                o2_ps = psum2.tile([P, F], mybir.dt.float32, space="PSUM")
                nc.tensor.matmul(out=o2_ps[:], lhsT=at_sb[:], rhs=w_sb[:], start=True, stop=True)
                nc.scalar.activation(outbuf[:, w * F : (w + 1) * F], o2_ps[:],
                                     mybir.ActivationFunctionType.Copy,
                                     scale=degt_sb[:, w : w + 1])
                # flush finished quarters of the output buffer so the
                # writeback overlaps compute instead of trailing serially
                if (w + 1) % (NW // 4) == 0 and w + 1 < NW:
                    nc.sync.dma_start(out=out_p[:, flushed * F : (w + 1) * F],
                                      in_=outbuf[:, flushed * F : (w + 1) * F])
                    flushed = w + 1
            nc.sync.dma_start(out=out_p[:, flushed * F :], in_=outbuf[:, flushed * F :])
    nc.compile()
    return nc


def _get_program(t_key):
    key = (t_key, tuple(GBT), SB_T, os.environ.get("GCN_PHASE", "3"))
    if key not in _PROGRAM_CACHE:
        _PROGRAM_CACHE[key] = _build_program(t_key)
    return _PROGRAM_CACHE[key]


def _run(nc, in_maps, trace=False, **kw):
    from concourse.bass_utils import run_bass_kernel_spmd

    return run_bass_kernel_spmd(nc, in_maps, core_ids=list(range(NCORES)),
                                trace=trace, **kw)


def kernel(X, weight, degrees, row_pointers, column_index, _trace=False, _ret_raw=False):
    assert X.shape == (N, F) and column_index.shape == (E,)
    xt, w16, idx_w, dst_t, degt, dest_of, t_key = _preprocess(
        X, weight, degrees, row_pointers, column_index
    )
    nc = _get_program(t_key)
    in_maps = []
    for c in range(NCORES):
        m = {"xt": xt, "degt": degt[c], "w16": w16}
        for q in range(NQ):
            m[f"idx{q}"] = idx_w[q][c]
            m[f"dstrel{q}"] = dst_t[q][c]
        in_maps.append(m)
    res = _run(nc, in_maps, trace=_trace)
    out = np.empty((N, F), np.float32)
    for c in range(NCORES):
        r = res.results[c]["out"].reshape(P, NW, F).transpose(1, 0, 2).reshape(NW * P, F)
        ids = dest_of[c]
        valid = ids >= 0
        out[ids[valid]] = r[valid]
    if _ret_raw:
        return out, res
    return out

